# revision 24
# baseline (speedup 1.0000x reference)
"""Multi-head "genetic" attention (windowed-causal, GQA) for Trainium2.

Self-contained: kernel(**inputs) takes full inputs, shards across 8
NeuronCores (2 query heads per core; value head h//4 per GQA), runs a
Bass/Tile kernel per core, and reduces the row-sharded output projection
partials on host.

v3 architecture:
- One merged pipeline for QKV proj + RMSNorm + scores + sigmoid stats.
  RMS sqrt is batched per 8 t-tiles so the scalar engine's activation
  table doesn't thrash between Sqrt and Sigmoid (a table load is 1.3us).
- All transposes ride the DMA XBAR (dma_start_transpose, issued from the
  sync engine): q/k head tiles, exp'd score strips, and the attn tile.
  No PE transposes, no diagonal-matmul fitness trick, no PSUM round trip
  for the transposed data.
- exp applies the gene-fitness scale as a per-partition activation scale
  directly on the bf16 score strips (softmax logits = fitness(t)*S(t,s)).
- Score/AV/out-projection matmuls all bf16 (fp32r matmuls with free dim
  < 256 run at 1/4 rate; bf16 also gets fast weight loads).
- RMS weights (and the 1/sqrt(hd) score scale) fold into the k side on
  the host: S = (q/|q|) . (k/|k|) * (wq*wk/8) per head-dim.

Shapes (hardcoded): x (1, 2048, 1024), H=16 heads, head_dim 64, HV=4
value heads, window 512 (causal band of 513).
"""

import numpy as np

import bass_rust
import concourse.bass as bass
import concourse.tile as tile
from concourse import mybir
from concourse.bass_utils import run_bass_kernel_spmd
from concourse.masks import make_identity

F32 = mybir.dt.float32
F32R = mybir.dt.float32r
BF16 = mybir.dt.bfloat16
AF = mybir.ActivationFunctionType
ALU = mybir.AluOpType

T, D, H, HD, HV, WIN = 2048, 1024, 16, 64, 4, 512
NCORES = 8
HPC = H // NCORES          # 2 heads per core
P = 128
TT = T // P                # 16 t-tiles
KT = D // P                # 8 k-tiles over d_model
QKW = HPC * HD             # 128 q (or k) columns per core
VW = HD                    # 64 v columns per core
QKVW = 2 * QKW + VW        # 320 fused projection columns
EPS = 1.1920929e-07
NB = WIN // P + 1          # 5 band s-tiles max
MASK_FILL = -1.0e6         # exp(fill * fitness) == 0 for any fitness here
GRP = 8                    # t-tiles per RMS-sqrt batch (act-table hygiene)

# ---------------------------------------------------------------------------
# This walrus build rejects >1 sem wait per instruction ("Too many sync wait
# commands"). Move extra waits onto same-engine NOPs inserted just before the
# offending instruction (engine queues are in-order, so blocking on the NOP
# is equivalent to blocking on the instruction itself).
_MAX_WAITS = 1


def split_multi_waits(nc, max_waits=_MAX_WAITS):
    for bb in nc.main_func.blocks:
        insts = bb.instructions
        i = 0
        while i < len(insts):
            inst = insts[i]
            si = inst.sync_info
            waits = list(si.on_wait or []) if si is not None else []
            if len(waits) > max_waits:
                si.on_wait = waits[-max_waits:]
                extra = waits[:-max_waits]
                nops = []
                for j in range(0, len(extra), max_waits):
                    n = nc.engines[inst.engine].nop(nofuse=True)
                    ni = n.ins
                    for bb2 in nc.main_func.blocks:
                        if ni in bb2.instructions:
                            bb2.instructions.remove(ni)
                            break
                    chunk = extra[j : j + max_waits]
                    if ni.sync_info is None:
                        ni.sync_info = bass_rust.SyncInfo(on_wait=chunk, on_update=[])
                    else:
                        ni.sync_info.on_wait = chunk
                    nops.append(ni)
                for k, ni in enumerate(nops):
                    insts.insert(i + k, ni)
                i += len(nops)
            i += 1
# ---------------------------------------------------------------------------


def _broadcast_row_ap(dram_ap, width):
    """DRAM AP replicating a (1, width) row across all 128 partitions."""
    return bass.AP(
        tensor=dram_ap.tensor,
        offset=dram_ap.offset,
        ap=[[0, P], [1, width]],
    )


def build_kernel(nc, tc, xT_d, wqkv_d, bqkv_d, rmswk_d, wo_d, out_d):
    from contextlib import ExitStack

    with ExitStack() as ctx:
        consts = ctx.enter_context(tc.tile_pool(name="consts", bufs=1))
        persist = ctx.enter_context(tc.tile_pool(name="persist", bufs=1))

        wqkv_sb = persist.tile([P, KT, QKVW], BF16)
        wo_sb = persist.tile([P, D], BF16)  # loaded later, before pass 2

        eps_t = consts.tile([P, 1], F32)
        nc.vector.memset(eps_t, EPS)
        ones_f = consts.tile([P, 1], F32)
        nc.vector.memset(ones_f, 1.0)
        ident_bf = consts.tile([P, P], BF16)
        make_identity(nc, ident_bf)

        # additive band masks (bf16): diag tile keeps s<=t, lead tile keeps
        # s>=t (within-tile coordinates)
        mask_diag = consts.tile([P, P], BF16)
        nc.gpsimd.memset(mask_diag, 0.0)
        nc.gpsimd.affine_select(
            out=mask_diag, in_=mask_diag, compare_op=ALU.is_ge, fill=MASK_FILL,
            base=0, pattern=[[-1, P]], channel_multiplier=1,
        )
        mask_lead = consts.tile([P, P], BF16)
        nc.gpsimd.memset(mask_lead, 0.0)
        nc.gpsimd.affine_select(
            out=mask_lead, in_=mask_lead, compare_op=ALU.is_ge, fill=MASK_FILL,
            base=0, pattern=[[1, P]], channel_multiplier=-1,
        )

        # 127 - p, used for the partial-band diagonal t-tiles (s_lo == 0)
        causal_cnt = consts.tile([P, P], F32)
        nc.gpsimd.memset(causal_cnt, 1.0)
        nc.gpsimd.affine_select(
            out=causal_cnt, in_=causal_cnt, compare_op=ALU.is_ge, fill=0.0,
            base=0, pattern=[[-1, P]], channel_multiplier=1,
        )
        corr_lt = consts.tile([P, 1], F32)
        nc.vector.reduce_sum(corr_lt, causal_cnt, axis=mybir.AxisListType.X)
        nc.vector.tensor_scalar(corr_lt, corr_lt, -1.0, 128.0, ALU.mult, ALU.add)

        # denominator offset per t-tile: rs/T + Cvec ; in-strip masked slots
        # produce sigmoid(-1e6)=0, so their 0.5 contribution moves here.
        cvec = {}
        for tt in range(min(NB - 1, TT)):
            W = (tt + 1) * P
            c_base = 0.5 * (T - W) / T + 0.5
            cv = consts.tile([P, 1], F32, tag=f"cvec{tt}")
            nc.vector.tensor_scalar(cv, corr_lt, 0.5 / T, c_base, ALU.mult, ALU.add)
            cvec[tt] = cv
        C_FULL = 0.5 * (T - NB * P + (P - 1)) / T + 0.5

        bias_b = consts.tile([P, QKVW], F32)
        nc.gpsimd.dma_start(bias_b, _broadcast_row_ap(bqkv_d[:], QKVW))
        rmswk_b = consts.tile([P, QKW], F32)
        nc.gpsimd.dma_start(rmswk_b, _broadcast_row_ap(rmswk_d[:], QKW))

        ones2 = consts.tile([2, P], F32)
        nc.vector.memset(ones2, 1.0)

        qT = persist.tile([P, T], BF16)     # rows: head0 dims 0-63, head1 64-127
        kT = persist.tile([P, T], BF16)
        qN = persist.tile([P, TT, QKW], BF16)   # normalized q, t-natural
        kN = persist.tile([P, TT, QKW], BF16)   # normalized+weighted k, t-natural
        vN = persist.tile([P, TT, VW + 2], BF16)  # v natural + ones col + pad
        recip_all = persist.tile([P, HPC, TT], F32)
        nc.vector.tensor_copy(
            vN[:, :, VW : VW + 2],
            ones_f[:, :, None].to_broadcast((P, TT, 2)),
        )
        fs_all = persist.tile([P, HPC, TT], F32)

        fill_zero = nc.gpsimd.to_reg(0.0)

        xT_t = xT_d.rearrange("(ko p) t -> p ko t", p=P)

        xTs_pre = {}
        for tt in range(2):
            xt = persist.tile([P, KT, P], BF16, tag=f"xpre{tt}")
            nc.sync.dma_start(xt, xT_t[:, :, tt * P : (tt + 1) * P])
            xTs_pre[tt] = xt
        nc.sync.dma_start(wqkv_sb, wqkv_d.rearrange("(ko p) n -> p ko n", p=P))

        strips = {}
        strip_pool = ctx.enter_context(tc.tile_pool(name="strips", bufs=1))

        # ------------- Merged pipeline: QKV proj (grouped RMS sqrt) +
        # q/k DMA transposes + banded scores + sigmoid stats
        with tc.tile_pool(name="m_sb", bufs=3) as m_sb, \
             tc.tile_pool(name="m_qkv", bufs=GRP + 1) as m_qkv, \
             tc.tile_pool(name="m_ps", bufs=2, space="PSUM") as m_ps, \
             tc.tile_pool(name="m_sc", bufs=3, space="PSUM") as m_sc:
            for g0 in range(0, TT, GRP):
                gn = min(GRP, TT - g0)
                ssum_g = m_sb.tile([P, GRP, 4], F32, tag="ssum", bufs=2)
                fac_g = m_sb.tile([P, GRP, 4], F32, tag="fac", bufs=2)
                rfac_g = m_sb.tile([P, GRP, 4], F32, tag="rfac", bufs=2)
                qkv_g = {}
                for tt in range(g0, g0 + gn):
                    gi = tt - g0
                    if tt in xTs_pre:
                        xTs = xTs_pre[tt]
                    else:
                        xTs = m_sb.tile([P, KT, P], BF16, tag="xT")
                        nc.sync.dma_start(xTs, xT_t[:, :, tt * P : (tt + 1) * P])

                    qkv_ps = m_ps.tile([P, QKVW], F32, tag="qkv")
                    for ko in range(KT):
                        nc.tensor.matmul(
                            qkv_ps, lhsT=xTs[:, ko, :], rhs=wqkv_sb[:, ko, :],
                            start=(ko == 0), stop=(ko == KT - 1),
                        )
                    qkv_sb = m_qkv.tile([P, QKVW], F32, tag="qkv_sb")
                    qkv_g[tt] = qkv_sb
                    nc.vector.tensor_add(qkv_sb, qkv_ps, bias_b)

                    sq = m_sb.tile([P, 2 * QKW], F32, tag="sq")
                    nc.gpsimd.tensor_mul(
                        sq, qkv_sb[:, : 2 * QKW], qkv_sb[:, : 2 * QKW]
                    )
                    nc.vector.reduce_sum(
                        ssum_g[:, gi, :], sq.rearrange("p (c d) -> p c d", d=HD),
                        axis=mybir.AxisListType.X,
                    )
                # one Sqrt for the whole group: scalar act table stays put
                nc.scalar.activation(
                    fac_g[:, :gn, :], ssum_g[:, :gn, :], AF.Sqrt,
                    bias=eps_t, scale=1.0 / HD,
                )
                nc.vector.reciprocal(rfac_g[:, :gn, :], fac_g[:, :gn, :])

                for tt in range(g0, g0 + gn):
                    gi = tt - g0
                    qkv_sb = qkv_g[tt]
                    qk = qkv_sb[:, : 2 * QKW].rearrange("p (c d) -> p c d", d=HD)
                    qn_t = qN[:, tt, :].rearrange("p (c d) -> p c d", d=HD)
                    kn_t = kN[:, tt, :].rearrange("p (c d) -> p c d", d=HD)
                    nc.gpsimd.tensor_tensor(
                        qn_t, qk[:, 0:2, :],
                        rfac_g[:, gi, 0:2, None].to_broadcast((P, 2, HD)), ALU.mult,
                    )
                    # rms weights (q*k product, pre-scaled 1/8) on the k side
                    nc.gpsimd.tensor_tensor(
                        kn_t, qk[:, 2:4, :],
                        rfac_g[:, gi, 2:4, None].to_broadcast((P, 2, HD)), ALU.mult,
                    )
                    nc.gpsimd.tensor_tensor(
                        kn_t, kn_t,
                        rmswk_b.rearrange("p (c d) -> p c d", d=HD), ALU.mult,
                    )
                    nc.gpsimd.tensor_copy(vN[:, tt, :VW], qkv_sb[:, 2 * QKW :])

                    # XBAR transposes: early small batches, then per-4
                    if tt in (1, 3, 7, 11, 15):
                        t4 = {1: 0, 3: 2, 7: 4, 11: 8, 15: 12}[tt]
                        nc.sync.dma_start_transpose(
                            qT[:, t4 * P : (tt + 1) * P].rearrange(
                                "p (i t) -> p i t", t=P
                            ),
                            qN[:, t4 : tt + 1, :],
                        )
                        nc.sync.dma_start_transpose(
                            kT[:, t4 * P : (tt + 1) * P].rearrange(
                                "p (i t) -> p i t", t=P
                            ),
                            kN[:, t4 : tt + 1, :],
                        )

                for tt in range(g0, g0 + gn):
                    # banded scores + sigmoid stats for both heads
                    s_lo = max(0, tt - (NB - 1))
                    nst = tt - s_lo + 1
                    W = nst * P
                    rs2 = m_sb.tile([P, 2], F32, tag="rs2")
                    for h in range(HPC):
                        ps = m_sc.tile([P, NB * P], F32, tag="S")
                        for c0 in range(0, W, 512):
                            cw = min(512, W - c0)
                            nc.tensor.matmul(
                                ps[:, c0 : c0 + cw],
                                lhsT=qT[h * HD : (h + 1) * HD,
                                        tt * P : (tt + 1) * P],
                                rhs=kT[h * HD : (h + 1) * HD,
                                       s_lo * P + c0 : s_lo * P + c0 + cw],
                                start=True, stop=True,
                            )
                        strip = strip_pool.tile([P, W], BF16, tag=f"st{h}_{tt}")
                        strips[(h, tt)] = strip
                        # copy psum->sbuf with band masks fused in (piecewise:
                        # same total elements, no separate masking pass)
                        mid_lo = P if nst == NB else 0
                        if mid_lo > 0:
                            nc.vector.tensor_tensor(
                                strip[:, :P], ps[:, :P], mask_lead, ALU.add
                            )
                        if W - P > mid_lo:
                            # h1's unmasked middle rides the scalar engine:
                            # Copy is in every act table (no reload) and the
                            # following sigmoid is on the same queue
                            if h == 0:
                                nc.vector.tensor_copy(
                                    strip[:, mid_lo : W - P], ps[:, mid_lo : W - P]
                                )
                            else:
                                nc.scalar.copy(
                                    strip[:, mid_lo : W - P], ps[:, mid_lo : W - P]
                                )
                        nc.vector.tensor_tensor(
                            strip[:, W - P : W], ps[:, W - P : W], mask_diag,
                            ALU.add,
                        )
                        sig = m_sb.tile([P, NB * P], F32, tag="sig")
                        nc.scalar.activation(
                            sig[:, :W], strip, AF.Sigmoid,
                            accum_out=rs2[:, h : h + 1],
                        )
                    den2 = m_sb.tile([P, 2], F32, tag="den2")
                    cv = cvec[tt] if nst < NB else None
                    if cv is not None:
                        nc.gpsimd.tensor_scalar(
                            den2, rs2, 1.0 / T, cv, ALU.mult, ALU.add
                        )
                    else:
                        nc.gpsimd.tensor_scalar(
                            den2, rs2, 1.0 / T, C_FULL, ALU.mult, ALU.add
                        )
                    nc.vector.reciprocal(recip_all[:, :, tt], den2)

        nc.sync.dma_start(wo_sb, wo_d[:])

        # gene fitness scale per (head, t): recip(t) / sum_t recip(t).
        # Cross-partition sum via PE ones-reduction, then an on-chip PE
        # broadcast of the two per-head scalars back across partitions.
        with tc.tile_pool(name="st_sb", bufs=1) as st_sb, \
             tc.tile_pool(name="st_ps", bufs=1, space="PSUM") as st_ps:
            rsum = st_sb.tile([P, HPC], F32, tag="rsum")
            nc.vector.reduce_sum(rsum, recip_all, axis=mybir.AxisListType.X)
            sinv_ps = st_ps.tile([HPC, 1], F32, tag="sp")
            nc.tensor.matmul(sinv_ps, lhsT=rsum, rhs=ones_f,
                             start=True, stop=True)
            sinv_r = st_sb.tile([HPC, 1], F32, tag="sinvr")
            nc.vector.reciprocal(sinv_r, sinv_ps)
            diag2 = st_sb.tile([HPC, HPC], F32, tag="diag2")
            nc.vector.tensor_copy(diag2, sinv_r.to_broadcast((HPC, HPC)))
            nc.gpsimd.affine_select(
                out=diag2, in_=diag2, compare_op=ALU.is_equal, fill=fill_zero,
                base=0, pattern=[[-1, HPC]], channel_multiplier=1,
            )
            srb_ps = st_ps.tile([P, HPC], F32, tag="srbp")
            nc.tensor.matmul(srb_ps, lhsT=ones2, rhs=diag2,
                             start=True, stop=True)
            srb = st_sb.tile([P, HPC], F32, tag="srb")
            nc.vector.tensor_copy(srb, srb_ps)
            nc.vector.tensor_tensor(
                fs_all, recip_all,
                srb[:, :, None].to_broadcast((P, HPC, TT)), ALU.mult,
            )

        # fitness diagonals: dmat = identity * fs (per-partition scalar), one
        # cheap vector op each, off pass 2's critical path
        dmat_pool = ctx.enter_context(tc.tile_pool(name="dmats", bufs=1))
        dmats = {}
        for tt in range(TT):
            for h in range(HPC):
                dm = dmat_pool.tile([P, P], BF16, tag=f"dm{h}_{tt}")
                nc.vector.tensor_scalar(
                    dm, ident_bf, fs_all[:, h, tt : tt + 1], None, ALU.mult
                )
                dmats[(h, tt)] = dm

        # ---------------- Pass 2: fitness-scaled eT via diag-matmul (fuses
        # the transpose), exp straight out of PSUM, AV, output projection.
        # All chains stay on-chip (PE->PSUM->scalar->PE), no DMA latency.
        with tc.tile_pool(name="p2_sb", bufs=4) as p2_sb, \
             tc.tile_pool(name="p2_wt", bufs=2, space="PSUM") as p2_wt, \
             tc.tile_pool(name="p2_wt2", bufs=1, space="PSUM") as p2_wt2, \
             tc.tile_pool(name="p2_av", bufs=2, space="PSUM") as p2_av, \
             tc.tile_pool(name="p2_at", bufs=1, space="PSUM") as p2_at, \
             tc.tile_pool(name="p2_o", bufs=2, space="PSUM") as p2_o:
            for tt in range(TT):
                s_lo = max(0, tt - (NB - 1))
                nst = tt - s_lo + 1
                attn = p2_sb.tile([P, QKW], BF16, tag="attn")
                for h in range(HPC):
                    dmat = dmats[(h, tt)]
                    av_ps = p2_av.tile([P, VW + 2], F32, tag="av")
                    eTs = []
                    st = 0
                    while st < nst:
                        pw = min(4, nst - st)
                        wt_ps = (p2_wt if st == 0 else p2_wt2).tile(
                            [P, 4, P], F32, tag="wt"
                        )
                        for k in range(pw):
                            nc.tensor.matmul(
                                wt_ps[:, k, :],
                                lhsT=strips[(h, tt)][:, (st + k) * P : (st + k + 1) * P],
                                rhs=dmat, start=True, stop=True,
                            )
                        eT = p2_sb.tile([P, 4, P], BF16, tag="eT", bufs=6)
                        nc.scalar.activation(
                            eT[:, :pw, :], wt_ps[:, :pw, :], AF.Exp
                        )
                        eTs.append((st, pw, eT))
                        st += pw
                    for st, pw, eT in eTs:
                        for k in range(pw):
                            nc.tensor.matmul(
                                av_ps, lhsT=eT[:, k, :],
                                rhs=vN[:, s_lo + st + k, :],
                                start=(st + k == 0), stop=(st + k == nst - 1),
                            )
                    erec = p2_sb.tile([P, 1], F32, tag="erec")
                    nc.vector.reciprocal(erec, av_ps[:, VW : VW + 1])
                    nc.vector.tensor_tensor(
                        attn[:, h * VW : (h + 1) * VW], av_ps[:, :VW],
                        erec.to_broadcast((P, VW)), ALU.mult,
                    )

                atp = p2_at.tile([P, P], BF16, tag="atp")
                nc.tensor.transpose(atp, attn, ident_bf)
                atT = p2_sb.tile([P, P], BF16, tag="atT")
                nc.vector.tensor_copy(atT, atp)
                out_sb = p2_sb.tile([P, D], F32, tag="osb")
                for ci, c0 in enumerate(range(0, D, 512)):
                    ops = p2_o.tile([P, 512], F32, tag="o")
                    nc.tensor.matmul(
                        ops, lhsT=atT, rhs=wo_sb[:, c0 : c0 + 512],
                        start=True, stop=True,
                    )
                    if ci == 0:
                        nc.vector.tensor_copy(out_sb[:, c0 : c0 + 512], ops)
                    else:
                        nc.scalar.copy(out_sb[:, c0 : c0 + 512], ops)
                nc.sync.dma_start(out_d[tt * P : (tt + 1) * P, :], out_sb)


def build_nc(repeats=1):
    nc = bass.Bass()
    xT_d = nc.declare_dram_parameter("xT", [D, T], BF16, isOutput=False)
    wqkv_d = nc.declare_dram_parameter("wqkv", [D, QKVW], BF16, isOutput=False)
    bqkv_d = nc.declare_dram_parameter("bqkv", [1, QKVW], F32, isOutput=False)
    rmswk_d = nc.declare_dram_parameter("rmswk", [1, QKW], F32, isOutput=False)
    wo_d = nc.declare_dram_parameter("wo", [QKW, D], BF16, isOutput=False)
    out_d = nc.declare_dram_parameter("out", [T, D], F32, isOutput=True)
    with tile.TileContext(nc) as tc:
        for _ in range(repeats):
            build_kernel(nc, tc, xT_d, wqkv_d, bqkv_d, rmswk_d, wo_d, out_d)
    split_multi_waits(nc)
    return nc


_NC_CACHE = None


def _get_nc():
    global _NC_CACHE
    if _NC_CACHE is None:
        _NC_CACHE = build_nc()
    return _NC_CACHE


def _to_bf16(a):
    import ml_dtypes

    return np.ascontiguousarray(np.asarray(a, np.float32).astype(ml_dtypes.bfloat16))


def make_in_maps(x, w_q, b_q, w_k, b_k, w_v, b_v, rms_q_w, rms_k_w, w_o):
    xT = _to_bf16(x.reshape(T, D).T)
    # fold rms_q_w * rms_k_w and the 1/sqrt(HD) score scale into the k side
    rk = (rms_q_w * rms_k_w / np.sqrt(HD)).astype(np.float32)
    rmswk = np.ascontiguousarray(np.concatenate([rk, rk])[None, :])
    in_maps = []
    for c in range(NCORES):
        qs = slice(c * QKW, (c + 1) * QKW)
        vs = slice((c // 2) * VW, (c // 2 + 1) * VW)
        wqkv = _to_bf16(
            np.concatenate([w_q[:, qs], w_k[:, qs], w_v[:, vs]], axis=1)
        )
        bqkv = np.ascontiguousarray(
            np.concatenate([b_q[qs], b_k[qs], b_v[vs]])[None, :]
        ).astype(np.float32)
        wo = _to_bf16(w_o[qs, :])
        in_maps.append(
            {"xT": xT, "wqkv": wqkv, "bqkv": bqkv, "rmswk": rmswk, "wo": wo}
        )
    return in_maps


def kernel(x, w_q, b_q, w_k, b_k, w_v, b_v, rms_q_w, rms_k_w, w_o, b_o, **kw):
    x = np.asarray(x, np.float32)
    args = [np.asarray(a, np.float32) for a in
            (w_q, b_q, w_k, b_k, w_v, b_v, rms_q_w, rms_k_w, w_o)]
    in_maps = make_in_maps(x, *args)
    nc = _get_nc()
    res = run_bass_kernel_spmd(nc, in_maps, core_ids=list(range(NCORES)), **kw)
    acc = np.zeros((T, D), np.float64)
    for c in range(NCORES):
        acc += res.results[c]["out"].astype(np.float64)
    out = (acc + np.asarray(b_o, np.float64)[None, :]).astype(np.float32)
    return out.reshape(1, T, D)
